# revision 28
# baseline (speedup 1.0000x reference)
"""Trainium2 Bass kernel for nn_ADConv (adaptive-basis conv).

Math (per image, per pixel q=(h,w)):
  h1  = tanh(bn1(conv3x3(x)))                      # [64, H, W]
  bc  = tanh(bn2(conv3x3(h1)))                     # [96, H, W], channel = 6f+t
  PB[c,t,q]   = sum_k x[c, q+dk] * B[t,k]          # depthwise basis conv
  u[c,f,q]    = sum_t PB[c,t,q] * bc[6f+t, wq, hq] # per-pixel bilinear (DVE)
  out[o,w,h]  = sum_{c,f} coef[o, 16c+f] * u[c,f,q]

Sharding: data-parallel, batch 16 -> 2 images per NeuronCore, params
replicated. Everything computed in bf16 (fp32 PSUM accumulation).

Structure (v3):
  - conv1/conv2 for both images first (dense PE stream)
  - theta partition layout p = t*64 + f*16 + c (t in 0..1 within a
    j3 pair, f in 0..3 within an f-block, c in 0..15 within a c-block):
    per block, 4 pbdup operand tiles (PB dup x4 over f) and 4 bcdup
    operand tiles (bc dup x16 over c) combine into 16 theta products
    covering all (c-block, f-block) pairs. Duplication is SBUF->SBUF
    DMA (no HBM round-trip), ~6MB per block vs 12.6MB DRAM ring in v2.
  - per (uc, img) block: PB matmuls -> pbt, dup DMAs, then 16 theta
    DVE muls ([128, 3072]) each feeding 6 PE matmuls into fps PSUM
  - software pipelining: PB+dup of block n+1 emitted before final of n
"""

import os
import sys

import numpy as np

sys.path.insert(0, "/opt/trn_rl_repo")

import ml_dtypes

import concourse.bacc as bacc
import concourse.bass as bass
import concourse.mybir as mybir
import concourse.tile as tile
from concourse.ap import AP
from concourse.bass_utils import run_bass_kernel_spmd

BF16 = mybir.dt.bfloat16
F32 = mybir.dt.float32
AF = mybir.ActivationFunctionType
ALU = mybir.AluOpType

N_CORES = 8
IMGS = 2           # images per core
C = 64             # input channels
INTER = 64         # conv1 out channels
BCH = 96           # conv2 out channels = 16f * 6t
NT = 6             # TOTAL_BASES
NF = 16            # NUM_FA
O = 128            # output channels
H = W = 64
HP = 66            # padded spatial
Q = H * W          # 4096 pixels
RC = 8             # rows per chunk
NCHUNK = H // RC   # 8 chunks of 512 px
CH = RC * W        # 512 px per chunk
UCH = 2 * CH       # 1024-px u-chunks
NUC = NCHUNK // 2  # 4 u-chunks
BN_EPS = 1e-5

_CACHE = {}


def build_graph():
    nc = bacc.Bacc(None, target_bir_lowering=False)

    xs = nc.declare_dram_parameter("xs", [IMGS, C, H, W], BF16, isOutput=False)
    w1p = nc.declare_dram_parameter("w1p", [128, 3, INTER], BF16, isOutput=False)
    w1s = nc.declare_dram_parameter("w1s", [C, 3, INTER], BF16, isOutput=False)
    s1 = nc.declare_dram_parameter("s1", [INTER, 1], F32, isOutput=False)
    b1 = nc.declare_dram_parameter("b1", [INTER, 1], F32, isOutput=False)
    w2p = nc.declare_dram_parameter("w2p", [128, 3, BCH], BF16, isOutput=False)
    w2s = nc.declare_dram_parameter("w2s", [INTER, 3, BCH], BF16, isOutput=False)
    s2 = nc.declare_dram_parameter("s2", [BCH, 1], F32, isOutput=False)
    b2 = nc.declare_dram_parameter("b2", [BCH, 1], F32, isOutput=False)
    wpbp = nc.declare_dram_parameter("wpbp", [128, 3, NT * C], BF16, isOutput=False)
    wpbs = nc.declare_dram_parameter("wpbs", [C, 3, NT * C], BF16, isOutput=False)
    coefp = nc.declare_dram_parameter("coefp", [128, 16, O], BF16, isOutput=False)
    out = nc.declare_dram_parameter("out", [IMGS, O, W, H], BF16, isOutput=True)

    with tile.TileContext(nc) as tc:
        with (
            tc.tile_pool(name="persist", bufs=1) as pp,
            tc.tile_pool(name="dup_pb", bufs=2) as dpb,
            tc.tile_pool(name="dup_bc", bufs=2) as dbc,
            tc.tile_pool(name="theta", bufs=3) as thp,
            tc.tile_pool(name="pbt", bufs=2) as pbp,
            tc.tile_pool(name="dramb", bufs=1, space=bass.MemorySpace.DRAM) as dp,
            tc.tile_pool(name="dramp", bufs=2, space=bass.MemorySpace.DRAM) as dpd,
            tc.tile_pool(name="ps_conv", bufs=2, space=bass.MemorySpace.PSUM) as pcv,
            tc.tile_pool(name="ps_pb", bufs=2, space=bass.MemorySpace.PSUM) as ppb,
            tc.tile_pool(name="ps_fin", bufs=2, space=bass.MemorySpace.PSUM) as pfn,
        ):
            # ---- persistent SBUF ----
            w1psb = pp.tile([128, 3, INTER], BF16, tag="w1psb")
            w1ssb = pp.tile([C, 3, INTER], BF16, tag="w1ssb")
            w2psb = pp.tile([128, 3, BCH], BF16, tag="w2psb")
            w2ssb = pp.tile([INTER, 3, BCH], BF16, tag="w2ssb")
            wpbpsb = pp.tile([128, 3, NT * C], BF16, tag="wpbpsb")
            wpbssb = pp.tile([C, 3, NT * C], BF16, tag="wpbssb")
            coefsb = pp.tile([128, 16, O], BF16, tag="coefsb")
            s1sb = pp.tile([INTER, 1], F32, tag="s1sb")
            b1sb = pp.tile([INTER, 1], F32, tag="b1sb")
            s2sb = pp.tile([BCH, 1], F32, tag="s2sb")
            b2sb = pp.tile([BCH, 1], F32, tag="b2sb")
            nc.sync.dma_start(w1psb[:], w1p[:])
            nc.sync.dma_start(w1ssb[:], w1s[:])
            nc.sync.dma_start(w2psb[:], w2p[:])
            nc.sync.dma_start(w2ssb[:], w2s[:])
            nc.gpsimd.dma_start(wpbpsb[:], wpbp[:])
            nc.gpsimd.dma_start(wpbssb[:], wpbs[:])
            nc.gpsimd.dma_start(coefsb[:], coefp[:])
            nc.scalar.dma_start(s1sb[:], s1[:])
            nc.scalar.dma_start(b1sb[:], b1[:])
            nc.scalar.dma_start(s2sb[:], s2[:])
            nc.scalar.dma_start(b2sb[:], b2[:])

            xpad = []
            hpad = []
            bcs = []
            outsb = []
            # DRAM staging for bc (only the x16 c-dup needs a DRAM
            # source: SBUF source APs cannot have 0-stride partitions).
            # Layout [uc, ch, q] so each (uc, 12-channel) window is one
            # contiguous region and the dup read merges to 3 AP dims.
            bcd = [
                dp.tile([NUC, BCH, UCH], BF16, tag=f"bcd{i}", name=f"bcd{i}")
                for i in range(IMGS)
            ]
            # DRAM staging for PB (same reason: the x4 f-dup read crosses
            # partitions as a function of a free index, which only a DRAM
            # source AP can express with partitions confined to dim0)
            pbd_pool = dpd
            for i in range(IMGS):
                xp = pp.tile([128, HP, HP], BF16, tag=f"xpad{i}", name=f"xpad{i}")
                hp = pp.tile([128, HP, HP], BF16, tag=f"hpad{i}", name=f"hpad{i}")
                bi = pp.tile([BCH, H, W], BF16, tag=f"bc{i}", name=f"bc{i}")
                nc.gpsimd.memset(xp[:], 0.0)
                nc.gpsimd.memset(hp[:], 0.0)
                nc.sync.dma_start(xp[0:64, 1 : H + 1, 1 : W + 1], xs[i])
                nc.scalar.dma_start(xp[64:128, 1 : H + 1, 0:W], xs[i])
                ob = pp.tile([O, W, H], BF16, tag=f"outsb{i}", name=f"outsb{i}")
                xpad.append(xp)
                hpad.append(hp)
                bcs.append(bi)
                outsb.append(ob)

            # ---- conv phase: conv1 both images, then conv2 both images ----
            for i in range(IMGS):
                for g in range(NCHUNK):
                    cpsf = pcv.tile([BCH, RC, W], F32, tag="convps")
                    cps = cpsf[0:INTER]
                    h0 = g * RC
                    for m in range(6):
                        ki = m % 3
                        pair = m < 3
                        lhsT = w1psb[:, ki, :] if pair else w1ssb[:, ki, :]
                        if pair:
                            rhs = xpad[i][:, h0 + ki : h0 + ki + RC, 0:W]
                        else:
                            rhs = xpad[i][0:64, h0 + ki : h0 + ki + RC, 2 : 2 + W]
                        nc.tensor.matmul(
                            cps, lhsT, rhs, start=(m == 0), stop=(m == 5)
                        )
                    nc.scalar.activation(
                        hpad[i][0:64, h0 + 1 : h0 + 1 + RC, 1 : W + 1],
                        cps,
                        AF.Tanh,
                        bias=b1sb[:],
                        scale=s1sb[:],
                    )
                    nc.scalar.activation(
                        hpad[i][64:128, h0 + 1 : h0 + 1 + RC, 0:W],
                        cps,
                        AF.Tanh,
                        bias=b1sb[:],
                        scale=s1sb[:],
                    )
            for i in range(IMGS):
                for g in range(NCHUNK):
                    cps = pcv.tile([BCH, RC, W], F32, tag="convps")
                    h0 = g * RC
                    for m in range(6):
                        ki = m % 3
                        pair = m < 3
                        lhsT = w2psb[:, ki, :] if pair else w2ssb[:, ki, :]
                        if pair:
                            rhs = hpad[i][:, h0 + ki : h0 + ki + RC, 0:W]
                        else:
                            rhs = hpad[i][0:64, h0 + ki : h0 + ki + RC, 2 : 2 + W]
                        nc.tensor.matmul(
                            cps[:], lhsT, rhs, start=(m == 0), stop=(m == 5)
                        )
                    # transposed store: bcs[ch, a, b] = conv2out[ch, b, a]
                    # so bcs free-order == PB pixel order q=(h, w).
                    # bcs partition r is the PERMUTED channel (see
                    # _prep_params): r = 24*fb + 12*t_l + 3*f_l + j3
                    # <-> original channel 6*(4*fb+f_l) + 2*j3 + t_l
                    nc.scalar.activation(
                        bcs[i][:, :, h0 : h0 + RC].transpose([0, 2, 1]),
                        cps[:],
                        AF.Tanh,
                        bias=b2sb[:],
                        scale=s2sb[:],
                    )
                # stage bc to DRAM (broadcast source for the c-dup)
                nc.sync.dma_start(
                    bcd[i][:].transpose([1, 0, 2]),
                    bcs[i][:].rearrange("p (u aa) b -> p u (aa b)", u=NUC),
                )

            # ---- final phase: per (uc, img) blocks, software-pipelined ----
            blocks = [(uc, i) for uc in range(NUC) for i in range(IMGS)]
            nb = len(blocks)
            pbt_tiles = {}
            dup_tiles = {}

            def emit_pb(bidx):
                # PB matmuls + ACT copies -> pbt tile, then duplication
                # DMAs (SBUF->SBUF) into pbdup/bcdup operand tiles.
                uc, i = blocks[bidx]
                pbt = pbp.tile([128, 3, UCH], BF16, tag="pbt", name=f"pbt{bidx}")
                pbt_tiles[bidx] = pbt
                for j3 in range(3):
                    for half in range(2):
                        h0 = (uc * 2 + half) * RC
                        pps = ppb.tile([128, RC, W], F32, tag="pbps", name="pps")
                        for m in range(6):
                            ki = m % 3
                            pair = m < 3
                            if pair:
                                lhsT = wpbpsb[:, ki, j3 * 128 : (j3 + 1) * 128]
                                rhs = xpad[i][:, h0 + ki : h0 + ki + RC, 0:W]
                            else:
                                lhsT = wpbssb[:, ki, j3 * 128 : (j3 + 1) * 128]
                                rhs = xpad[i][0:64, h0 + ki : h0 + ki + RC, 2 : 2 + W]
                            nc.tensor.matmul(
                                pps[:], lhsT, rhs, start=(m == 0), stop=(m == 5)
                            )
                        nc.scalar.copy(
                            pbt[:, j3, half * CH : (half + 1) * CH],
                            pps[:].opt(),
                        )
                # duplication DMAs
                pbdup = dpb.tile([128, 4, 3, UCH], BF16, tag="pbdup", name=f"pbdup{bidx}")
                bcdup = dbc.tile([128, 4, 3, UCH], BF16, tag="bcdup", name=f"bcdup{bidx}")
                dup_tiles[bidx] = (pbdup, bcdup)
                qi = 0
                # stage PB to DRAM for the dup read
                pbd = pbd_pool.tile([128, 3 * UCH], BF16, tag="pbd", name=f"pbd{bidx}")
                nc.sync.dma_start(pbd[:], pbt[:].rearrange("p j q -> p (j q)"))
                # PB dup x4 over f: dst rows t*64 + f*16 + c  <-  pbd row
                # t*64 + 16*cb + c, free (cb, j3q). One dma per (t_l, f_l);
                # dst partitions confined to dim0 (16-row range).
                for t_l in range(2):
                    srch = pbd[t_l * 64 : (t_l + 1) * 64].rearrange(
                        "(cb c) m -> c cb m", cb=4
                    )
                    for f_l in range(4):
                        p0 = t_l * 64 + f_l * 16
                        dst = pbdup[p0 : p0 + 16].rearrange(
                            "c cb j q -> c cb (j q)"
                        )
                        iss = (nc.sync, nc.scalar)[qi % 2]
                        qi += 1
                        iss.dma_start(dst, srch)
                # bc dup x16 over c: dst rows t*64 + f*16 + c  <-  bcd
                # row 24*fb + 12*t_l + 3*f_l + j3 in the uc window. One
                # dma per (t_l, f_l): dst [16, fb, j3q], src rows
                # (12*t_l+3*f_l)+{0,1,2} within each 24-row fb group,
                # 0-stride dup dim outermost.
                for t_l in range(2):
                    srcg = bcd[i][uc].rearrange("(fb g) q -> fb (g q)", fb=4)
                    for f_l in range(4):
                        off = (12 * t_l + 3 * f_l) * UCH
                        p0 = t_l * 64 + f_l * 16
                        dst = bcdup[p0 : p0 + 16].rearrange(
                            "c fb j q -> c fb (j q)"
                        )
                        src = srcg[:, off : off + 3 * UCH].partition_broadcast(
                            16
                        )
                        iss = (nc.sync, nc.scalar)[qi % 2]
                        qi += 1
                        iss.dma_start(dst, src)

            def emit_final(bidx):
                uc, i = blocks[bidx]
                pbt_tiles.pop(bidx)
                pbdup, bcdup = dup_tiles.pop(bidx)
                fps = pfn.tile([O, 2, CH], F32, tag="finps", name=f"fps{bidx}")
                for kk in range(16):
                    cb, fb = kk // 4, kk % 4
                    theta = thp.tile([128, 3, UCH], BF16, tag="theta", name="theta")
                    nc.vector.tensor_mul(
                        theta[:], pbdup[:, cb].opt(), bcdup[:, fb].opt()
                    )
                    for j3 in range(3):
                        for half in range(2):
                            nc.tensor.matmul(
                                fps[:, half],
                                coefsb[:, kk, :],
                                theta[:, j3, half * CH : (half + 1) * CH],
                                start=(kk == 0 and j3 == 0),
                                stop=(kk == 15 and j3 == 2),
                            )
                for half in range(2):
                    ch = uc * 2 + half
                    h0 = ch * RC
                    nc.scalar.copy(
                        outsb[i][:, :, h0 : h0 + RC].transpose([0, 2, 1]),
                        fps[:, half].opt(),
                    )

            # pipelined emission: PB+dup one block ahead of final
            emit_pb(0)
            for b in range(nb):
                if b + 1 < nb:
                    emit_pb(b + 1)
                emit_final(b)
            # single big output store per image (128 x 8KB descriptors)
            for i in range(IMGS):
                iss = (nc.sync, nc.scalar)[i % 2]
                iss.dma_start(out[i], outsb[i][:])

    nc.compile()
    return nc


def _prep_params(inputs):
    bf16 = ml_dtypes.bfloat16
    f32 = np.float32
    c1w = np.asarray(inputs["conv1_w"], f32)
    c2w = np.asarray(inputs["conv2_w"], f32)
    bases = np.asarray(inputs["bases"], f32)
    coef = np.asarray(inputs["coef"], f32)

    s1 = np.asarray(inputs["bn1_gamma"], f32) / np.sqrt(
        np.asarray(inputs["bn1_var"], f32) + BN_EPS
    )
    b1 = (np.asarray(inputs["conv1_b"], f32) - np.asarray(inputs["bn1_mean"], f32)) * s1 + np.asarray(
        inputs["bn1_beta"], f32
    )
    s2 = np.asarray(inputs["bn2_gamma"], f32) / np.sqrt(
        np.asarray(inputs["bn2_var"], f32) + BN_EPS
    )
    b2 = (np.asarray(inputs["conv2_b"], f32) - np.asarray(inputs["bn2_mean"], f32)) * s2 + np.asarray(
        inputs["bn2_beta"], f32
    )

    # conv2 output-channel permutation: bcs row r = 24*fb + 12*t_l +
    # 3*f_l + j3 holds original channel 6*(4*fb + f_l) + 2*j3 + t_l
    perm = np.empty(BCH, np.int64)
    for r in range(BCH):
        fb = r // 24
        rem = r % 24
        t_l = rem // 12
        k = rem % 12
        f_l = k // 3
        j3 = k % 3
        perm[r] = 6 * (4 * fb + f_l) + 2 * j3 + t_l
    c2wp = c2w[perm]
    s2 = s2[perm]
    b2 = b2[perm]

    w1pk = np.zeros((128, 3, INTER), f32)
    w1sk = np.zeros((C, 3, INTER), f32)
    w2pk = np.zeros((128, 3, BCH), f32)
    w2sk = np.zeros((INTER, 3, BCH), f32)
    for ki in range(3):
        w1pk[0:64, ki] = c1w[:, :, ki, 0].T
        w1pk[64:128, ki] = c1w[:, :, ki, 1].T
        w1sk[:, ki] = c1w[:, :, ki, 2].T
        w2pk[0:64, ki] = c2wp[:, :, ki, 0].T
        w2pk[64:128, ki] = c2wp[:, :, ki, 1].T
        w2sk[:, ki] = c2wp[:, :, ki, 2].T

    wpbpk = np.zeros((128, 3, NT * C), f32)
    wpbsk = np.zeros((C, 3, NT * C), f32)
    for t in range(NT):
        for c in range(C):
            for ki in range(3):
                wpbpk[c, ki, t * C + c] = bases[t, 3 * ki + 0]
                wpbpk[64 + c, ki, t * C + c] = bases[t, 3 * ki + 1]
                wpbsk[c, ki, t * C + c] = bases[t, 3 * ki + 2]

    # coefT[p, kk=(cb*4+fb), o] = coef[o, 16*(16*cb+c_l) + 4*fb + f_l]
    # for p = t_l*64 + f_l*16 + c_l (independent of t_l).
    cview = coef.reshape(O, C, NF)  # coef[o, c, f]
    coefT = np.zeros((128, 16, O), f32)
    for cb in range(4):
        for fb in range(4):
            kk = cb * 4 + fb
            for f_l in range(4):
                for c_l in range(16):
                    row = cview[:, 16 * cb + c_l, 4 * fb + f_l]  # [O]
                    coefT[f_l * 16 + c_l, kk] = row
                    coefT[64 + f_l * 16 + c_l, kk] = row

    return {
        "w1p": w1pk.astype(bf16),
        "w1s": w1sk.astype(bf16),
        "s1": s1.reshape(-1, 1).astype(f32),
        "b1": b1.reshape(-1, 1).astype(f32),
        "w2p": w2pk.astype(bf16),
        "w2s": w2sk.astype(bf16),
        "s2": s2.reshape(-1, 1).astype(f32),
        "b2": b2.reshape(-1, 1).astype(f32),
        "wpbp": wpbpk.astype(bf16),
        "wpbs": wpbsk.astype(bf16),
        "coefp": coefT.astype(bf16),
    }


def kernel(**inputs):
    if "nc" not in _CACHE:
        _CACHE["nc"] = build_graph()
    nc = _CACHE["nc"]

    params = _prep_params(inputs)
    x = np.asarray(inputs["x"], np.float32).astype(ml_dtypes.bfloat16)

    in_maps = []
    for core in range(N_CORES):
        m = dict(params)
        m["xs"] = np.ascontiguousarray(x[core * IMGS : (core + 1) * IMGS])
        in_maps.append(m)

    res = run_bass_kernel_spmd(nc, in_maps, core_ids=list(range(N_CORES)))
    outs = [r["out"] for r in res.results]
    return np.concatenate(outs, axis=0).astype(np.float32)


def _install_ntff_hook():
    """Shim antenv.axon_hooks with the trn_boot ctypes NTFF hook."""
    import types

    try:
        from antenv.axon_hooks import get_axon_ntff_profile_hook  # noqa
        return
    except ImportError:
        pass
    sys.path.insert(0, "/root/.axon_site/trn_agent_boot")
    import trn_boot

    hook = trn_boot._ntff_profile_via_ctypes("/opt/axon/libaxon_pjrt.so")
    mod_pkg = sys.modules.get("antenv")
    if mod_pkg is None:
        mod_pkg = types.ModuleType("antenv")
        sys.modules["antenv"] = mod_pkg
    mod = types.ModuleType("antenv.axon_hooks")
    mod.get_axon_ntff_profile_hook = lambda: hook
    mod.set_axon_ntff_profile_hook = lambda h: None
    sys.modules["antenv.axon_hooks"] = mod
    mod_pkg.axon_hooks = mod


def run_timed(inputs):
    """Run once with NTFF tracing; return exec_time_ns (or None)."""
    _install_ntff_hook()
    if "nc" not in _CACHE:
        _CACHE["nc"] = build_graph()
    nc = _CACHE["nc"]
    params = _prep_params(inputs)
    x = np.asarray(inputs["x"], np.float32).astype(ml_dtypes.bfloat16)
    in_maps = []
    for core in range(N_CORES):
        m = dict(params)
        m["xs"] = np.ascontiguousarray(x[core * IMGS : (core + 1) * IMGS])
        in_maps.append(m)
    res = run_bass_kernel_spmd(
        nc, in_maps, core_ids=list(range(N_CORES)), trace=True
    )
    print("trace profile_json:", res.profile_json)
    _CACHE["last_res"] = res
    return res.exec_time_ns


if __name__ == "__main__":
    rng = np.random.default_rng(0)
    fake = {
        "x": rng.standard_normal((16, 64, 64, 64)).astype(np.float32),
        "conv1_w": (rng.standard_normal((64, 64, 3, 3)) * 0.05).astype(np.float32),
        "conv1_b": (rng.standard_normal((64,)) * 0.05).astype(np.float32),
        "bn1_gamma": rng.uniform(0.5, 1.5, (64,)).astype(np.float32),
        "bn1_beta": (rng.standard_normal((64,)) * 0.05).astype(np.float32),
        "bn1_mean": (rng.standard_normal((64,)) * 0.05).astype(np.float32),
        "bn1_var": rng.uniform(0.5, 1.5, (64,)).astype(np.float32),
        "conv2_w": (rng.standard_normal((96, 64, 3, 3)) * 0.05).astype(np.float32),
        "conv2_b": (rng.standard_normal((96,)) * 0.05).astype(np.float32),
        "bn2_gamma": rng.uniform(0.5, 1.5, (96,)).astype(np.float32),
        "bn2_beta": (rng.standard_normal((96,)) * 0.05).astype(np.float32),
        "bn2_mean": (rng.standard_normal((96,)) * 0.05).astype(np.float32),
        "bn2_var": rng.uniform(0.5, 1.5, (96,)).astype(np.float32),
        "bases": rng.standard_normal((6, 9)).astype(np.float32),
        "coef": (rng.standard_normal((128, 1024)) * 0.02).astype(np.float32),
    }
    o = kernel(**fake)
    print("out", o.shape, o.dtype)
